# revision 1
# baseline (speedup 1.0000x reference)
"""Trainium2 Bass kernel for nn_BaseGraphEncoder (gnn_message_passing).

Computation (per batch b):
    h0 = max_k x[:, idx[b,n,k]]            (gather + K-max, "local maxpool")
    h1 = h0 @ W1 + b1
    r  = relu(Wc1 @ h1 + bc1)              (conv1d k=1)
    h2 = max_k r[:, idx[b,n,k]]            (second local maxpool, same idx)
    h3 = h2 @ W2 + b2
    out = Wc2 @ h3 + bc2                   (conv1d k=1, 1024 out channels)

Sharding: data-parallel over batch B=16 across 8 NeuronCores (2 batches/core).
The kNN gather is batch-local so no cross-core communication is needed.

Device strategy per batch:
  - The gather is done with gpsimd.dma_gather (SWDGE indexed DMA): the host
    pre-transposes x to point-major rows (N, 64) f32 = 256B rows in DRAM and
    converts idx to batch-local int16 lists in k-major order.  k-major makes
    the K-max reduce into elementwise tensor_tensor(max) ops on DVE over
    the gather output tiles.
  - PE transposes (via identity matmul) convert point-major <-> feature-major.
  - All matmuls in fp32 on the tensor engine (N=512 per matmul / PSUM bank).
  - conv1 output r is transposed back to point-major rows and written to a
    DRAM scratch so the second dma_gather can fetch it.
  - conv2 bias is pre-loaded into PSUM with a K=1 matmul against a ones row,
    so the PSUM->SBUF copy needs no bias op and can go on any idle engine.
"""

import sys

if "/opt/trn_rl_repo" not in sys.path:
    sys.path.insert(0, "/opt/trn_rl_repo")

import numpy as np

import concourse.bacc as bacc
import concourse.bass as bass
import concourse.mybir as mybir
import concourse.tile as tile
from concourse._compat import get_trn_type
from concourse.bass_utils import run_bass_kernel_spmd

B, C, N, K = 16, 64, 2048, 16
NCORES = 8
BPC = B // NCORES  # batches per core
D1, D2, DOUT = 64, 128, 1024  # hidden dims
NIDX = N * K  # 32768 gather indices per batch per layer
# single_packet=False is essential on this runtime: it both lifts the
# ~1024-index SWDGE ring cap and runs ~6x faster (HW-measured 16.6 ns/idx
# at 8192 indices/instruction vs ~100 ns/idx with single_packet=True).
NCHUNK = 4  # default; _build_nc can override
CH_IDX = NIDX // NCHUNK  # 8192 indices per chunk
F32 = mybir.dt.float32
I16 = mybir.dt.int16

_compiled = None


def _build_nc(reps=1, nchunk=None, gbufs=3):
    """Build the kernel. reps>1 wraps the body in a dynamic For_i loop —
    used only for wall-clock differential timing (same output each rep)."""
    NCHUNK = nchunk or 4
    CH_IDX = NIDX // NCHUNK
    nc = bacc.Bacc(get_trn_type() or "TRN2", target_bir_lowering=False, debug=False)

    xt_d = nc.dram_tensor("xt", [BPC, N, C], F32, kind="ExternalInput")
    idx_d = nc.dram_tensor("idx16", [BPC, 128, NIDX // 16], I16, kind="ExternalInput")
    w1_d = nc.dram_tensor("W1", [C, D1], F32, kind="ExternalInput")
    wc1t_d = nc.dram_tensor("Wc1T", [D1, D1], F32, kind="ExternalInput")
    w2_d = nc.dram_tensor("W2", [D1, D2], F32, kind="ExternalInput")
    wc2t_d = nc.dram_tensor("Wc2T", [D2, DOUT], F32, kind="ExternalInput")
    b1_d = nc.dram_tensor("b1", [D1, 1], F32, kind="ExternalInput")
    bc1_d = nc.dram_tensor("bc1", [D1, 1], F32, kind="ExternalInput")
    b2_d = nc.dram_tensor("b2", [D2, 1], F32, kind="ExternalInput")
    bc2_d = nc.dram_tensor("bc2", [1, DOUT], F32, kind="ExternalInput")
    id_d = nc.dram_tensor("ident", [128, 128], F32, kind="ExternalInput")
    out_d = nc.dram_tensor("out", [BPC, DOUT, N], F32, kind="ExternalOutput")

    with tile.TileContext(nc) as tc:
        with (
            tc.tile_pool(name="consts", bufs=1) as consts,
            tc.tile_pool(name="gpool", bufs=gbufs) as gpool,
            tc.tile_pool(name="accpool", bufs=2) as accpool,
            tc.tile_pool(name="idxpool", bufs=2) as idxpool,
            tc.tile_pool(name="featpool", bufs=2) as featpool,
            tc.tile_pool(name="h3pool", bufs=2) as h3pool,
            tc.tile_pool(name="rpmpool", bufs=2) as rpmpool,
            tc.tile_pool(name="outpool", bufs=3) as outpool,
            tc.tile_pool(name="pst", bufs=3, space="PSUM") as pst,
            tc.tile_pool(name="psm", bufs=2, space="PSUM") as psm,
            tc.tile_pool(name="pso", bufs=2, space="PSUM") as pso,
            tc.tile_pool(name="drampool", bufs=2, space="DRAM") as drampool,
        ):
            # ---- constants ----
            w1_sb = consts.tile([C, D1], F32, tag="w1")
            wc1t_sb = consts.tile([D1, D1], F32, tag="wc1t")
            w2_sb = consts.tile([D1, D2], F32, tag="w2")
            wc2t_sb = consts.tile([D2, DOUT], F32, tag="wc2t")
            b1_sb = consts.tile([D1, 1], F32, tag="b1")
            bc1_sb = consts.tile([D1, 1], F32, tag="bc1")
            b2_sb = consts.tile([D2, 1], F32, tag="b2")
            bc2_sb = consts.tile([1, DOUT], F32, tag="bc2")
            id_sb = consts.tile([128, 128], F32, tag="ident")
            ones_sb = consts.tile([1, 512], F32, tag="ones")
            nc.sync.dma_start(w1_sb, w1_d[:])
            nc.sync.dma_start(wc1t_sb, wc1t_d[:])
            nc.sync.dma_start(w2_sb, w2_d[:])
            nc.sync.dma_start(wc2t_sb, wc2t_d[:])
            nc.sync.dma_start(b1_sb, b1_d[:])
            nc.sync.dma_start(bc1_sb, bc1_d[:])
            nc.sync.dma_start(b2_sb, b2_d[:])
            nc.sync.dma_start(bc2_sb, bc2_d[:])
            nc.sync.dma_start(id_sb, id_d[:])
            nc.vector.memset(ones_sb, 1.0)

            def gather_max_layer(src_dram_ap, idx_sb):
                """dma_gather K=16 neighbor rows (k-major) + DVE max.

                Returns point-major SBUF tile acc[128, 1024] (2D view of
                [128, 16, 64]): acc[p, q*64:(q+1)*64] = max-pooled features
                of point n = q*128 + p.

                With k-major index order and 8192-index chunks, chunk ch
                holds neighbors j = 4*ch .. 4*ch+3 for all n:
                g2[:, jj*1024:(jj+1)*1024] is one full point-major copy.
                """
                acc = accpool.tile([128, (N // 128) * C], F32, tag="acc")
                for ch in range(NCHUNK):
                    # gather needs the 3D [128, nidx/128, elem] view
                    g = gpool.tile([128, CH_IDX // 128, C], F32, tag="g")
                    nc.gpsimd.dma_gather(
                        g,
                        src_dram_ap,
                        idx_sb[:, ch * (CH_IDX // 16) : (ch + 1) * (CH_IDX // 16)],
                        CH_IDX,
                        CH_IDX,
                        C,
                        single_packet=False,
                    )
                    g2 = g.rearrange("p a c -> p (a c)")
                    for jj in range(CH_IDX // N):
                        gsl = g2[:, jj * 1024 : (jj + 1) * 1024]
                        if ch == 0 and jj == 0:
                            continue
                        elif ch == 0 and jj == 1:
                            nc.vector.tensor_tensor(
                                acc, g2[:, 0:1024], gsl, mybir.AluOpType.max
                            )
                        else:
                            nc.vector.tensor_tensor(acc, acc, gsl, mybir.AluOpType.max)
                return acc

            def transpose_pm_to_fm(acc, tagsfx):
                """point-major acc[128, 1024] -> feature-major [64, 2048]."""
                fm = featpool.tile([C, N], F32, tag="fm" + tagsfx)
                for q4 in range(4):
                    pt = pst.tile([128, 512], F32, tag="pt")
                    for qq in range(4):
                        q = q4 * 4 + qq
                        nc.tensor.transpose(
                            pt[:C, qq * 128 : (qq + 1) * 128],
                            acc[:, q * C : (q + 1) * C],
                            id_sb,
                        )
                    nc.vector.tensor_copy(fm[:, q4 * 512 : (q4 + 1) * 512], pt[:C, :])
                return fm

            def body(_iv=None):
                emit_batches()

            def emit_batches():
                for b in range(BPC):
                    emit_batch(b)

            def emit_batch(b):
                idx_sb = idxpool.tile([128, NIDX // 16], I16, tag="idx")
                nc.sync.dma_start(idx_sb, idx_d[b])

                # ---------- layer 1: gather+max over x ----------
                acc1 = gather_max_layer(xt_d[b], idx_sb)
                h0T = transpose_pm_to_fm(acc1, "0")

                # ---------- linear1 + bias ----------
                h1 = featpool.tile([D1, N], F32, tag="h1")
                for m in range(4):
                    pm = psm.tile([128, 512], F32, tag="pm")
                    nc.tensor.matmul(pm[:D1, :], w1_sb, h0T[:, m * 512 : (m + 1) * 512])
                    nc.scalar.activation(
                        h1[:, m * 512 : (m + 1) * 512],
                        pm[:D1, :],
                        mybir.ActivationFunctionType.Identity,
                        bias=b1_sb,
                    )

                # ---------- conv1 + bias + relu ----------
                r = featpool.tile([D1, N], F32, tag="r")
                for m in range(4):
                    pm = psm.tile([128, 512], F32, tag="pm")
                    nc.tensor.matmul(pm[:D1, :], wc1t_sb, h1[:, m * 512 : (m + 1) * 512])
                    nc.scalar.activation(
                        r[:, m * 512 : (m + 1) * 512],
                        pm[:D1, :],
                        mybir.ActivationFunctionType.Relu,
                        bias=bc1_sb,
                    )

                # ---------- transpose r to point-major rows, write scratch ----------
                r_pm = rpmpool.tile([128, (N // 128) * C], F32, tag="rpm")
                for q4 in range(4):
                    pt = pst.tile([128, 512], F32, tag="pt")
                    for qq in range(4):
                        q = q4 * 4 + qq
                        nc.tensor.transpose(
                            pt[:, qq * C : (qq + 1) * C],
                            r[:, q * 128 : (q + 1) * 128],
                            id_sb[:C, :C],
                        )
                    nc.vector.tensor_copy(
                        r_pm[:, q4 * 256 : (q4 + 1) * 256], pt[:, :256]
                    )
                rt = drampool.tile([N // 128, 128, C], F32, tag="rt")
                # rt[q, p, :] = r_pm[p, q*64:(q+1)*64]  -> DRAM row n = q*128+p
                nc.sync.dma_start(
                    rt.rearrange("q p c -> p q c"),
                    r_pm.rearrange("p (q c) -> p q c", c=C),
                )

                # ---------- layer 2: gather+max over r ----------
                acc2 = gather_max_layer(rt.rearrange("q p c -> (q p) c"), idx_sb)
                h2T = transpose_pm_to_fm(acc2, "2")

                # ---------- linear2 + bias ----------
                h3 = h3pool.tile([D2, N], F32, tag="h3")
                for m in range(4):
                    pm = psm.tile([128, 512], F32, tag="pm")
                    nc.tensor.matmul(pm, w2_sb, h2T[:, m * 512 : (m + 1) * 512])
                    nc.scalar.activation(
                        h3[:, m * 512 : (m + 1) * 512],
                        pm,
                        mybir.ActivationFunctionType.Identity,
                        bias=b2_sb,
                    )

                # ---------- conv2 (1024 out channels) + bias ----------
                for dc in range(8):
                    osb = outpool.tile([128, N], F32, tag="osb")
                    for m in range(4):
                        po = pso.tile([128, 512], F32, tag="po")
                        # bias row -> PSUM via K=1 matmul, then accumulate conv2
                        nc.tensor.matmul(
                            po,
                            bc2_sb[:, dc * 128 : (dc + 1) * 128],
                            ones_sb,
                            start=True,
                            stop=False,
                        )
                        nc.tensor.matmul(
                            po,
                            wc2t_sb[:, dc * 128 : (dc + 1) * 128],
                            h3[:, m * 512 : (m + 1) * 512],
                            start=False,
                            stop=True,
                        )
                        nc.any.tensor_copy(osb[:, m * 512 : (m + 1) * 512], po)
                    nc.sync.dma_start(out_d[b, dc * 128 : (dc + 1) * 128, :], osb)

            if reps == 1:
                emit_batches()
            else:
                with tc.For_i(0, reps, 1):
                    emit_batches()

    nc.compile()
    return nc


def _get_nc():
    global _compiled
    if _compiled is None:
        _compiled = _build_nc()
    return _compiled


def _prep_inputs(x, idx, W1, b1, Wc1, bc1, W2, b2, Wc2, bc2):
    """Host-side sharding + layout marshalling -> per-core in_maps."""
    x = np.asarray(x, np.float32)
    idx = np.asarray(idx)
    xt = np.ascontiguousarray(x.transpose(0, 2, 1))  # (B, N, C) point-major rows

    # batch-local indices (reference guarantees idx[b] in [b*N, (b+1)*N))
    local = idx.astype(np.int64) - (np.arange(B, dtype=np.int64) * N)[:, None, None]
    assert local.min() >= 0 and local.max() < N, "idx not batch-local"
    local = local.astype(np.int16)  # (B, N, K)

    # k-major flat list L[j*N + n] = local[b, n, j], wrapped: W[p, s] = L[s*16+p]
    km = local.transpose(0, 2, 1).reshape(B, NIDX)  # (B, K*N)
    wrapped = km.reshape(B, NIDX // 16, 16).transpose(0, 2, 1)  # (B, 16, NIDX/16)
    wrapped = np.ascontiguousarray(
        np.tile(wrapped, (1, 8, 1))
    )  # replicate to 128 partitions

    common = {
        "W1": np.ascontiguousarray(np.asarray(W1, np.float32)),
        "Wc1T": np.ascontiguousarray(np.asarray(Wc1, np.float32).T),
        "W2": np.ascontiguousarray(np.asarray(W2, np.float32)),
        "Wc2T": np.ascontiguousarray(np.asarray(Wc2, np.float32).T),
        "b1": np.asarray(b1, np.float32).reshape(D1, 1),
        "bc1": np.asarray(bc1, np.float32).reshape(D1, 1),
        "b2": np.asarray(b2, np.float32).reshape(D2, 1),
        "bc2": np.asarray(bc2, np.float32).reshape(1, DOUT),
        "ident": np.eye(128, dtype=np.float32),
    }
    in_maps = []
    for c in range(NCORES):
        bs = [BPC * c + j for j in range(BPC)]
        m = dict(common)
        m["xt"] = np.ascontiguousarray(xt[bs])
        m["idx16"] = np.ascontiguousarray(wrapped[bs])
        in_maps.append(m)
    return in_maps


def kernel(_trace=False, _trace_kwargs=None, **inputs):
    nc = _get_nc()
    in_maps = _prep_inputs(**inputs)
    res = run_bass_kernel_spmd(
        nc,
        in_maps,
        list(range(NCORES)),
        trace=_trace,
        **(_trace_kwargs or {}),
    )
    out = np.empty((B, DOUT, N), np.float32)
    for c in range(NCORES):
        for j in range(BPC):
            out[BPC * c + j] = res.results[c]["out"][j]
    if _trace:
        return out, res
    return out

